# revision 19
# baseline (speedup 1.0000x reference)
"""Expert-parallel MoE SwiGLU kernel for one TRN2 chip (8 NeuronCores).

Problem: out[n] = sum_k w[n,k] * FFN_{idx[n,k]}(x[n]) with E=8 experts,
top-2 routing, H=1024, I=4096, N=2048 tokens.

Strategy: one expert per core. Tokens are routed (gathered) per expert on
the host, each core runs the three bf16 matmuls of its expert's SwiGLU FFN
(silu(x@w1) * (x@w3)) @ w2 over its token batch entirely transposed
(tokens along the PE moving/free dim), and the host scatter-adds the
returned per-expert outputs with the routing weights. Tokens whose two
routing slots hit the SAME expert are deduplicated on the host (weights
summed, FFN computed once). Expert token counts above the per-core
capacity CAP spill to a small host-side f32 pass so the device runs a
single full-width chunk.

Schedule notes (from NTFF profile analysis): the kernel is tensor-bound
(768 bf16 matmuls of C moving columns at ~0.42 ns/col; fp8 is ruled out
by the 2e-2 relative-error budget — measured 5.7% rel err vs bf16's
0.48%). Startup: the engine preamble runs ~7.3us, then the DMA rings
start (sync ring's first bytes ~0.8us after trigger, scalar ~3.2us,
gpsimd ~4.3us). Ring arbitration under multi-way contention is
winner-take-most and varies run to run, so the early window is a strict
2-way split: sync carries the ii=0 block in its solo head start
(~11.3us), then x in two kh-halves, then pairs j0/j2; scalar carries
the ii=1..3 singles FIFO-serialized then pair j1; gpsimd stays dark
until ~30us. The ii=0/ii=1 chains run as open-psum half-chains
interleaved around the x-hi arrival so the PE goes dense from x-lo
(~13.5us) instead of waiting for all of x. Warmup matmuls ramp the HAM
clock and keep the PE busy until the gate lands (a >3us PE idle gap
triggers a HAM down-throttle to half clock that costs 5-20us). All
remaining weight traffic (1MB w13 pair tiles, then w2 pair tiles)
shares one tile-pool tag (bufs=3) whose buffer reuse paces every DMA
trigger behind pair j-3's chain consumption — without this the
scheduler fires dependency-free triggers during startup and floods the
critical window (w2 tiles were observed transferring at ~14us, 100us
before Phase B needs them). Pairs rotate gpsimd/sync/scalar from j3. y
is returned as bf16 (halves the store traffic; +0.4% quadrature error,
well within budget) and the final output chunk is split 240/240 (both
halves above the ~230-col LDWEIGHTS-bound threshold) with the last
piece on the scalar ring to shorten the kernel tail.
"""

import sys

for _p in ("/opt/trn_rl_repo", "/opt/pypackages"):
    if _p not in sys.path:
        sys.path.insert(0, _p)

import numpy as np
import ml_dtypes

import concourse.tile as tile
from concourse import bacc, mybir
from concourse.bass_utils import run_bass_kernel_spmd

P = 128
H = 1024
I = 4096
KH = H // P    # 8 contraction subtiles for the first matmuls
II = I // P    # 32 intermediate subtiles / contraction subtiles for w2
CAP = 480      # per-core token capacity (single PE moving chunk)
# PE warmup matmuls: ramp the HAM clock AND keep the tensor engine busy
# until the startup-critical DMAs are fully resident. Overshoot costs
# ~107ns/matmul; undershoot risks a HAM re-throttle that halves the
# early real matmul rate.
N_WARM = 28
W_COLS = 256   # warmup matmul moving width (finer tail granularity)
TAIL = 240     # final output piece width (short kernel tail). Keep the
               # final chain's moving width >= ~230 cols: below that the
               # 32-matmul chain goes LDWEIGHTS-bound (~97ns/matmul floor)
               # and the split costs more PE time than the tail it saves.

BF16 = mybir.dt.bfloat16
F32 = mybir.dt.float32


def _build(C):
    """One-expert SwiGLU FFN over C tokens (C <= 512), transposed layout.

    DRAM inputs (per core):
      xg    [P, KH, C]          bf16  x^T: [hp, kh, c] = x[tok c, kh*P+hp]
      w013t [P, 2, KH, P]       bf16  ii=0 block: [hp, 0, kh, m] = w1,
            [hp, 1, kh, m] = w3 (one 4KB-line transfer on scalar)
      w13st [3, P, 2, KH, P]    bf16  ii=1..3 singles (sync ring):
            [i, hp, 0, kh, m] = w1[kh*P+hp, (i+1)*P+m], [_,1,_] = w3
      w13pt [14, P, 2, 2, KH, P] bf16, pair-major ii blocks (ii=4..31):
            [j, hp, a, 0, kh, m] = w1[kh*P+hp, (2j+4+a)*P+m], [...,1,...] = w3
      w2t   [KH/2, P, 2, II, P] bf16, pair-major hh blocks:
            [q, ip, b, ik, m] = w2[ik*P+ip, (2q+b)*P+m]
    Output:
      yt    [KH/2, P, 2, C]     bf16  y^T, pair-major output subtiles
    """
    assert C <= 512
    nc = bacc.Bacc("TRN2", target_bir_lowering=False, debug=False)
    xg = nc.dram_tensor("xg", [P, KH, C], BF16, kind="ExternalInput")
    w013t = nc.dram_tensor("w013t", [P, 2, KH, P], BF16, kind="ExternalInput")
    w13st = nc.dram_tensor(
        "w13st", [3, P, 2, KH, P], BF16, kind="ExternalInput"
    )
    w13pt = nc.dram_tensor(
        "w13pt", [(II - 4) // 2, P, 2, 2, KH, P], BF16, kind="ExternalInput"
    )
    w2t = nc.dram_tensor(
        "w2t", [KH // 2, P, 2, II, P], BF16, kind="ExternalInput"
    )
    yt = nc.dram_tensor("yt", [KH // 2, P, 2, C], BF16, kind="ExternalOutput")

    with tile.TileContext(nc) as tc:
        with (
            tc.tile_pool(name="xp", bufs=2) as xp,
            tc.tile_pool(name="pp", bufs=1) as pp,
            tc.tile_pool(name="wp", bufs=8) as wp,
            tc.tile_pool(name="gp", bufs=4) as gp,
            tc.tile_pool(name="yp", bufs=3) as yp,
            tc.tile_pool(name="warm", bufs=1) as warm,
            tc.tile_pool(name="psA", bufs=3, space="PSUM") as psA,
            tc.tile_pool(name="psB", bufs=2, space="PSUM") as psB,
        ):
            # Startup-critical loads, all on sync (first ring up, ~8.7us):
            # the ii=0 weight block rides the solo window (lands ~11.3us),
            # then x in two kh-halves (~13.5/~16us) so the prologue chains
            # can start on x-lo. high_priority pins these triggers at the
            # head of the engine queue.
            with tc.high_priority():
                w013sb = wp.tile([P, 2, KH, P], BF16, tag="w0", bufs=1)
                nc.sync.dma_start(w013sb[:], w013t[:])
                xlo = xp.tile([P, KH // 2, C], BF16)
                nc.sync.dma_start(xlo[:], xg[:, : KH // 2, :])
                xhi = xp.tile([P, KH // 2, C], BF16)
                nc.sync.dma_start(xhi[:], xg[:, KH // 2 :, :])

            def xh(kh):
                return xlo[:, kh, :] if kh < KH // 2 else xhi[:, kh - KH // 2, :]

            # PE warmup: ramp the tensor engine to high-activity clock while
            # the input DMAs are in flight. Reads a zeroed tile, result is
            # never consumed.
            wtile = warm.tile([P, W_COLS], BF16)
            nc.vector.memset(wtile[:], 0.0)
            # Shares the Phase B psum pool (tag "py"): warmup is long done
            # before Phase B allocates its first chain psum.
            wps = psB.tile([P, W_COLS], F32, tag="py")
            for i in range(N_WARM):
                nc.tensor.matmul(
                    wps, wtile[:, :P], wtile[:], start=(i == 0),
                    stop=(i == N_WARM - 1),
                )

            psb = pp.tile([P, II, C], BF16)

            # Phase A: h1 = silu(x@w1), h3 = x@w3, p = h1*h3 (all transposed)
            # Prologue: ii=0 and ii=1 run as open-psum half-chains
            # interleaved around the x-hi arrival: [u0,g0,u1,g1] over
            # kh 0..3 as soon as w013+x-lo land (~12.5us), then the same
            # over kh 4..7 once x-hi lands (~15.2us). Keeps the PE dense
            # from x-lo onward instead of waiting for all of x.
            ws1 = wp.tile([P, 2, KH, P], BF16, tag="w13", bufs=3)
            nc.scalar.dma_start(ws1[:], w13st[0])
            psel = [
                lambda half, kh: w013sb[:, half, kh, :],
                lambda half, kh: ws1[:, half, kh, :],
            ]
            pg0 = psA.tile([P, C], F32, tag="pg")
            pg1 = psA.tile([P, C], F32, tag="pg")
            pu0 = psA.tile([P, C], F32, tag="pu")
            pu1 = psA.tile([P, C], F32, tag="pu")
            ppg = [pg0, pg1]
            ppu = [pu0, pu1]
            for lo in (0, KH // 2):
                for pii in (0, 1):
                    for ps, h in ((ppu[pii], 1), (ppg[pii], 0)):
                        for kh in range(lo, lo + KH // 2):
                            nc.tensor.matmul(
                                ps,
                                psel[pii](h, kh),
                                xh(kh),
                                start=(kh == 0),
                                stop=(kh == KH - 1),
                            )
            for pii in (0, 1):
                gs = gp.tile([P, C], BF16, tag="g")
                nc.scalar.activation(
                    gs, ppg[pii], mybir.ActivationFunctionType.Silu
                )
                nc.vector.tensor_tensor(
                    psb[:, pii, :], gs, ppu[pii], mybir.AluOpType.mult
                )

            wpair = None
            for ii in range(2, II):
                if ii in (2, 3):
                    # Singles ride scalar FIFO-serialized (2-way early
                    # split: sync carries w013+x+j0/j2, scalar the
                    # singles+j1; gpsimd stays dark until ~30us).
                    wsb = wp.tile([P, 2, KH, P], BF16, tag="w13", bufs=3)
                    nc.scalar.dma_start(wsb[:], w13st[ii - 1])
                    wsel = lambda half, kh, t=wsb: t[:, half, kh, :]
                elif (ii - 4) % 2 == 0:
                    j = (ii - 4) // 2
                    wpair = wp.tile([P, 2, 2, KH, P], BF16, tag="w13p", bufs=3)
                    # j0/j1 behind the ii=0 block on scalar, j2 behind the
                    # singles on sync; from j3 the pool tag paces triggers
                    # (trigger j waits until pair j-3's chains complete) and
                    # the three rings rotate.
                    if j in (0, 2):
                        eng = nc.sync
                    elif j == 1:
                        eng = nc.scalar
                    else:
                        eng = (nc.gpsimd, nc.scalar, nc.sync)[(j - 3) % 3]
                    eng.dma_start(wpair[:], w13pt[j])
                    wsel = lambda half, kh, t=wpair: t[:, 0, half, kh, :]
                else:
                    wsel = lambda half, kh, t=wpair: t[:, 1, half, kh, :]
                pg = psA.tile([P, C], F32, tag="pg")
                pu = psA.tile([P, C], F32, tag="pu")
                halves = (0, 1)
                for half in halves:
                    ps = pg if half == 0 else pu
                    for kh in range(KH):
                        nc.tensor.matmul(
                            ps,
                            wsel(half, kh),
                            xh(kh),
                            start=(kh == 0),
                            stop=(kh == KH - 1),
                        )
                gs = gp.tile([P, C], BF16, tag="g")
                nc.scalar.activation(gs, pg, mybir.ActivationFunctionType.Silu)
                nc.vector.tensor_tensor(
                    psb[:, ii, :], gs, pu, mybir.AluOpType.mult
                )

            # Phase B: y = p @ w2 (transposed: yT = w2T-contraction over I).
            # w2 arrives as 1MB pair tiles (hh 2q, 2q+1 together) rotated
            # across gpsimd/sync/scalar (gpsimd's trigger stream runs ahead
            # of compute, so q=0 prefetches early) and y goes out as bf16
            # pair stores. The last hh is split column-wise so its first
            # piece's copy+DMA overlaps the final piece's matmuls (shorter
            # kernel tail), with the final small piece on the scalar ring.
            w2sb = yd = None
            for hh in range(KH):
                q, b = divmod(hh, 2)
                if b == 0:
                    # w2 tiles share the pairs' pool tag: a bare-pool w2
                    # trigger has no dependencies and the scheduler fires it
                    # during STARTUP (~14us), stealing the critical window's
                    # HBM bandwidth. Sharing the tag paces each w2 trigger
                    # behind pair j-consumption (~101us+, still >10us before
                    # Phase B needs it).
                    w2sb = wp.tile([P, 2, II, P], BF16, tag="w13p", bufs=3)
                    eng = (nc.gpsimd, nc.scalar, nc.gpsimd, nc.scalar)[q]
                    eng.dma_start(w2sb[:], w2t[q])
                    yd = yp.tile([P, 2, C], BF16, tag="y2")
                halves = [(0, C)] if hh < KH - 1 or C <= TAIL else [
                    (0, C - TAIL), (C - TAIL, TAIL),
                ]
                for hi, (c0, cc) in enumerate(halves):
                    py = psB.tile([P, cc], F32, tag="py")
                    for ik in range(II):
                        nc.tensor.matmul(
                            py,
                            w2sb[:, b, ik, :],
                            psb[:, ik, c0 : c0 + cc],
                            start=(ik == 0),
                            stop=(ik == II - 1),
                        )
                    # DVE copies keep the COPY activation table off the
                    # scalar queue (its ACT_TABLE_LOAD would delay the scalar
                    # DMA ring's startup-critical triggers by ~1.3us).
                    if hh < KH - 1 or hi == 0:
                        nc.vector.tensor_copy(yd[:, b, c0 : c0 + cc], py)
                        if b == 1 and hh < KH - 1:
                            nc.sync.dma_start(yt[q], yd[:])
                        elif hh == KH - 1:
                            # penultimate store: hh=6 whole + hh=7 first
                            # piece, one trigger
                            nc.sync.dma_start(
                                yt[q, :, 0, :], yd[:, 0, :]
                            )
                            nc.sync.dma_start(
                                yt[q, :, 1, c0 : c0 + cc],
                                yd[:, 1, c0 : c0 + cc],
                            )
                    else:
                        # Final piece on the other hardware DGE ring,
                        # pipelined behind the first piece's store.
                        yb = yp.tile([P, cc], BF16, tag="y")
                        nc.vector.tensor_copy(yb, py)
                        nc.scalar.dma_start(
                            yt[q, :, 1, c0 : c0 + cc], yb[:]
                        )

    nc.compile()
    return nc


_PROGRAM_CACHE = {}


def _host_swiglu(x, w1e, w2e, w3e):
    g = x @ w1e
    u = x @ w3e
    g = g / (1.0 + np.exp(-g))
    return (g * u) @ w2e


def kernel(x, expert_indices, expert_weights, w1, w2, w3):
    x = np.asarray(x, dtype=np.float32)
    idx = np.asarray(expert_indices)
    wts = np.asarray(expert_weights, dtype=np.float32)
    w1 = np.asarray(w1, dtype=np.float32)
    w2 = np.asarray(w2, dtype=np.float32)
    w3 = np.asarray(w3, dtype=np.float32)
    N = x.shape[0]
    E = w1.shape[0]
    K = idx.shape[1]
    bf16 = ml_dtypes.bfloat16

    # host-side routing with dedup: a token whose routing slots both hit
    # expert e is computed once with the slot weights summed (exact:
    # (w0+w1)*FFN = w0*FFN + w1*FFN). Tokens beyond CAP spill to the host
    # f32 path (tiny tail, keeps device at one full-width PE chunk).
    toks, tokw, spill_toks, spill_w = [], [], [], []
    for e in range(E):
        hit = idx == e  # [N, K]
        rows = np.nonzero(hit.any(axis=1))[0]
        w_e = (wts[rows] * hit[rows]).sum(axis=1)
        toks.append(rows[:CAP])
        tokw.append(w_e[:CAP])
        spill_toks.append(rows[CAP:])
        spill_w.append(w_e[CAP:])
    C = max(16, max(len(t) for t in toks))
    C = ((C + 7) // 8) * 8

    if C not in _PROGRAM_CACHE:
        _PROGRAM_CACHE[C] = _build(C)
    nc = _PROGRAM_CACHE[C]

    in_maps = []
    for e in range(E):
        xt = np.zeros((C, H), dtype=np.float32)
        if len(toks[e]):
            xt[: len(toks[e])] = x[toks[e]]
        # [C, H] -> [hp, kh, c]
        xge = xt.T.reshape(KH, P, C).transpose(1, 0, 2)
        # w1/w3 [H, I] -> [ii, hp, {w1,w3}, kh, m]
        w13 = np.stack(
            [
                w1[e].reshape(KH, P, II, P).transpose(2, 1, 0, 3),
                w3[e].reshape(KH, P, II, P).transpose(2, 1, 0, 3),
            ],
            axis=2,
        )  # [II, P, 2, KH, P]
        w13 = w13.astype(bf16)
        # pair-major pairs for ii>=4: [II/2-2, P, 2, 2, KH, P]
        w13p = np.ascontiguousarray(
            w13[4:].reshape((II - 4) // 2, 2, P, 2, KH, P).swapaxes(1, 2)
        )
        in_maps.append(
            {
                "xg": np.ascontiguousarray(xge.astype(bf16)),
                "w013t": np.ascontiguousarray(w13[0]),
                "w13st": np.ascontiguousarray(w13[1:4]),
                "w13pt": w13p,
                "w2t": np.ascontiguousarray(
                    w2[e].reshape(II, P, KH, P).transpose(2, 1, 0, 3)
                    .reshape(KH // 2, 2, P, II, P).swapaxes(1, 2).astype(bf16)
                ),
            }
        )

    res = run_bass_kernel_spmd(nc, in_maps, core_ids=list(range(E)))

    out = np.zeros((N, H), dtype=np.float32)
    for e in range(E):
        cnt = len(toks[e])
        if cnt:
            y = (
                res.results[e]["yt"]
                .astype(np.float32)
                .reshape(KH // 2, P, 2, C)
                .swapaxes(1, 2)
                .reshape(H, C)
                .T[:cnt]
            )
            np.add.at(out, toks[e], y * tokw[e][:, None])
        if len(spill_toks[e]):
            ys = _host_swiglu(x[spill_toks[e]], w1[e], w2[e], w3[e])
            np.add.at(out, spill_toks[e], ys * spill_w[e][:, None])
    return out


# revision 20
# speedup vs baseline: 1.1836x; 1.1836x over previous
"""Expert-parallel MoE SwiGLU kernel for one TRN2 chip (8 NeuronCores).

Problem: out[n] = sum_k w[n,k] * FFN_{idx[n,k]}(x[n]) with E=8 experts,
top-2 routing, H=1024, I=4096, N=2048 tokens.

Strategy: one expert per core. Tokens are routed (gathered) per expert on
the host, each core runs the three bf16 matmuls of its expert's SwiGLU FFN
(silu(x@w1) * (x@w3)) @ w2 over its token batch entirely transposed
(tokens along the PE moving/free dim), and the host scatter-adds the
returned per-expert outputs with the routing weights. Tokens whose two
routing slots hit the SAME expert are deduplicated on the host (weights
summed, FFN computed once). Expert token counts above the per-core
capacity CAP spill to a small host-side f32 pass so the device runs a
single full-width chunk.

Schedule notes (from NTFF profile analysis): the kernel is tensor-bound
(768 bf16 matmuls of C moving columns at ~0.42 ns/col; fp8 is ruled out
by the 2e-2 relative-error budget — measured 5.7% rel err vs bf16's
0.48%). Startup: the engine preamble runs ~7.3us, then the DMA rings
start (sync ring's first bytes ~0.8us after trigger, scalar ~3.2us,
gpsimd ~4.3us). Ring arbitration under multi-way contention is
winner-take-most and varies run to run, so the early window is a strict
2-way split: sync carries the ii=0 block in its solo head start
(~11.3us), then x in two kh-halves, then pairs j0/j2; scalar carries
the ii=1..3 singles FIFO-serialized then pair j1; gpsimd stays dark
until ~30us. The ii=0/ii=1 chains run as open-psum half-chains
interleaved around the x-hi arrival so the PE goes dense from x-lo
(~13.5us) instead of waiting for all of x. Warmup matmuls ramp the HAM
clock and keep the PE busy until the gate lands (a >3us PE idle gap
triggers a HAM down-throttle to half clock that costs 5-20us). All
remaining weight traffic (1MB w13 pair tiles, then w2 pair tiles)
shares one tile-pool tag (bufs=3) whose buffer reuse paces every DMA
trigger behind pair j-3's chain consumption — without this the
scheduler fires dependency-free triggers during startup and floods the
critical window (w2 tiles were observed transferring at ~14us, 100us
before Phase B needs them). Pairs rotate gpsimd/sync/scalar from j3. y
is returned as bf16 (halves the store traffic; +0.4% quadrature error,
well within budget) and the final output chunk is split 240/240 (both
halves above the ~230-col LDWEIGHTS-bound threshold) with the last
piece on the scalar ring to shorten the kernel tail.
"""

import sys

for _p in ("/opt/trn_rl_repo", "/opt/pypackages"):
    if _p not in sys.path:
        sys.path.insert(0, _p)

import numpy as np
import ml_dtypes

import concourse.tile as tile
from concourse import bacc, mybir
from concourse.bass_utils import run_bass_kernel_spmd

P = 128
H = 1024
I = 4096
KH = H // P    # 8 contraction subtiles for the first matmuls
II = I // P    # 32 intermediate subtiles / contraction subtiles for w2
CAP = 480      # per-core token capacity (single PE moving chunk)
# PE warmup matmuls: ramp the HAM clock AND keep the tensor engine busy
# until the startup-critical DMAs are fully resident. Overshoot costs
# ~107ns/matmul; undershoot risks a HAM re-throttle that halves the
# early real matmul rate.
N_WARM = 28
W_COLS = 256   # warmup matmul moving width (finer tail granularity)
TAIL = 240     # final output piece width (short kernel tail). Keep the
               # final chain's moving width >= ~230 cols: below that the
               # 32-matmul chain goes LDWEIGHTS-bound (~97ns/matmul floor)
               # and the split costs more PE time than the tail it saves.

BF16 = mybir.dt.bfloat16
F32 = mybir.dt.float32


def _build(C):
    """One-expert SwiGLU FFN over C tokens (C <= 512), transposed layout.

    DRAM inputs (per core):
      xg    [P, KH, C]          bf16  x^T: [hp, kh, c] = x[tok c, kh*P+hp]
      w013t [P, 2, KH, P]       bf16  ii=0 block: [hp, 0, kh, m] = w1,
            [hp, 1, kh, m] = w3 (one 4KB-line transfer on scalar)
      w13st [3, P, 2, KH, P]    bf16  ii=1..3 singles (sync ring):
            [i, hp, 0, kh, m] = w1[kh*P+hp, (i+1)*P+m], [_,1,_] = w3
      w13pt [14, P, 2, 2, KH, P] bf16, pair-major ii blocks (ii=4..31):
            [j, hp, a, 0, kh, m] = w1[kh*P+hp, (2j+4+a)*P+m], [...,1,...] = w3
      w2t   [KH/2, P, 2, II, P] bf16, pair-major hh blocks:
            [q, ip, b, ik, m] = w2[ik*P+ip, (2q+b)*P+m]
    Output:
      yt    [KH/2, P, 2, C]     bf16  y^T, pair-major output subtiles
    """
    assert C <= 512
    nc = bacc.Bacc("TRN2", target_bir_lowering=False, debug=False)
    xg = nc.dram_tensor("xg", [P, KH, C], BF16, kind="ExternalInput")
    w013t = nc.dram_tensor("w013t", [P, 2, KH, P], BF16, kind="ExternalInput")
    w13st = nc.dram_tensor(
        "w13st", [3, P, 2, KH, P], BF16, kind="ExternalInput"
    )
    w13pt = nc.dram_tensor(
        "w13pt", [(II - 4) // 2, P, 2, 2, KH, P], BF16, kind="ExternalInput"
    )
    w2t = nc.dram_tensor(
        "w2t", [KH // 2, P, 2, II, P], BF16, kind="ExternalInput"
    )
    yt = nc.dram_tensor("yt", [KH // 2, P, 2, C], BF16, kind="ExternalOutput")

    with tile.TileContext(nc) as tc:
        with (
            tc.tile_pool(name="xp", bufs=2) as xp,
            tc.tile_pool(name="pp", bufs=1) as pp,
            tc.tile_pool(name="wp", bufs=8) as wp,
            tc.tile_pool(name="gp", bufs=4) as gp,
            tc.tile_pool(name="yp", bufs=3) as yp,
            tc.tile_pool(name="warm", bufs=1) as warm,
            tc.tile_pool(name="psA", bufs=3, space="PSUM") as psA,
            tc.tile_pool(name="psB", bufs=2, space="PSUM") as psB,
        ):
            # Startup-critical loads, all on sync (first ring up, ~8.7us):
            # the ii=0 weight block rides the solo window (lands ~11.3us),
            # then x in two kh-halves (~13.5/~16us) so the prologue chains
            # can start on x-lo. high_priority pins these triggers at the
            # head of the engine queue.
            with tc.high_priority():
                w013sb = wp.tile([P, 2, KH, P], BF16, tag="w0", bufs=1)
                nc.sync.dma_start(w013sb[:], w013t[:])
                xlo = xp.tile([P, KH // 2, C], BF16)
                nc.sync.dma_start(xlo[:], xg[:, : KH // 2, :])
                xhi = xp.tile([P, KH // 2, C], BF16)
                nc.sync.dma_start(xhi[:], xg[:, KH // 2 :, :])

            def xh(kh):
                return xlo[:, kh, :] if kh < KH // 2 else xhi[:, kh - KH // 2, :]

            # PE warmup: ramp the tensor engine to high-activity clock while
            # the input DMAs are in flight. Reads a zeroed tile, result is
            # never consumed.
            wtile = warm.tile([P, W_COLS], BF16)
            nc.vector.memset(wtile[:], 0.0)
            # Shares the Phase B psum pool (tag "py"): warmup is long done
            # before Phase B allocates its first chain psum.
            wps = psB.tile([P, W_COLS], F32, tag="py")
            for i in range(N_WARM):
                nc.tensor.matmul(
                    wps, wtile[:, :P], wtile[:], start=(i == 0),
                    stop=(i == N_WARM - 1),
                )

            psb = pp.tile([P, II, C], BF16)

            # Phase A: h1 = silu(x@w1), h3 = x@w3, p = h1*h3 (all transposed)
            # Prologue: ii=0 and ii=1 run as open-psum half-chains
            # interleaved around the x-hi arrival: [u0,g0,u1,g1] over
            # kh 0..3 as soon as w013+x-lo land (~12.5us), then the same
            # over kh 4..7 once x-hi lands (~15.2us). Keeps the PE dense
            # from x-lo onward instead of waiting for all of x.
            ws1 = wp.tile([P, 2, KH, P], BF16, tag="w13", bufs=3)
            nc.scalar.dma_start(ws1[:], w13st[0])
            psel = [
                lambda half, kh: w013sb[:, half, kh, :],
                lambda half, kh: ws1[:, half, kh, :],
            ]
            pg0 = psA.tile([P, C], F32, tag="pg")
            pg1 = psA.tile([P, C], F32, tag="pg")
            pu0 = psA.tile([P, C], F32, tag="pu")
            pu1 = psA.tile([P, C], F32, tag="pu")
            ppg = [pg0, pg1]
            ppu = [pu0, pu1]
            for lo in (0, KH // 2):
                for pii in (0, 1):
                    for ps, h in ((ppu[pii], 1), (ppg[pii], 0)):
                        for kh in range(lo, lo + KH // 2):
                            nc.tensor.matmul(
                                ps,
                                psel[pii](h, kh),
                                xh(kh),
                                start=(kh == 0),
                                stop=(kh == KH - 1),
                            )
            for pii in (0, 1):
                gs = gp.tile([P, C], BF16, tag="g")
                nc.scalar.activation(
                    gs, ppg[pii], mybir.ActivationFunctionType.Silu
                )
                nc.vector.tensor_tensor(
                    psb[:, pii, :], gs, ppu[pii], mybir.AluOpType.mult
                )

            wpair = None
            for ii in range(2, II):
                if ii in (2, 3):
                    # Singles ride scalar FIFO-serialized (2-way early
                    # split: sync carries w013+x+j0/j2, scalar the
                    # singles+j1; gpsimd stays dark until ~30us).
                    wsb = wp.tile([P, 2, KH, P], BF16, tag="w13", bufs=3)
                    nc.scalar.dma_start(wsb[:], w13st[ii - 1])
                    wsel = lambda half, kh, t=wsb: t[:, half, kh, :]
                elif (ii - 4) % 2 == 0:
                    j = (ii - 4) // 2
                    wpair = wp.tile([P, 2, 2, KH, P], BF16, tag="w13p", bufs=3)
                    # j0/j1 behind the ii=0 block on scalar, j2 behind the
                    # singles on sync; from j3 the pool tag paces triggers
                    # (trigger j waits until pair j-3's chains complete) and
                    # the three rings rotate.
                    if j < 3:
                        # all three early pairs behind x on sync: keeps
                        # scalar's early pull to just the 1.5MB of singles
                        # so the sync gate stream (w013+x halves) gets the
                        # larger HBM share.
                        eng = nc.sync
                    else:
                        eng = (nc.gpsimd, nc.scalar, nc.sync)[(j - 3) % 3]
                    eng.dma_start(wpair[:], w13pt[j])
                    wsel = lambda half, kh, t=wpair: t[:, 0, half, kh, :]
                else:
                    wsel = lambda half, kh, t=wpair: t[:, 1, half, kh, :]
                pg = psA.tile([P, C], F32, tag="pg")
                pu = psA.tile([P, C], F32, tag="pu")
                halves = (0, 1)
                for half in halves:
                    ps = pg if half == 0 else pu
                    for kh in range(KH):
                        nc.tensor.matmul(
                            ps,
                            wsel(half, kh),
                            xh(kh),
                            start=(kh == 0),
                            stop=(kh == KH - 1),
                        )
                gs = gp.tile([P, C], BF16, tag="g")
                nc.scalar.activation(gs, pg, mybir.ActivationFunctionType.Silu)
                nc.vector.tensor_tensor(
                    psb[:, ii, :], gs, pu, mybir.AluOpType.mult
                )

            # Phase B: y = p @ w2 (transposed: yT = w2T-contraction over I).
            # w2 arrives as 1MB pair tiles (hh 2q, 2q+1 together) rotated
            # across gpsimd/sync/scalar (gpsimd's trigger stream runs ahead
            # of compute, so q=0 prefetches early) and y goes out as bf16
            # pair stores. The last hh is split column-wise so its first
            # piece's copy+DMA overlaps the final piece's matmuls (shorter
            # kernel tail), with the final small piece on the scalar ring.
            w2sb = yd = None
            for hh in range(KH):
                q, b = divmod(hh, 2)
                if b == 0:
                    # w2 tiles share the pairs' pool tag: a bare-pool w2
                    # trigger has no dependencies and the scheduler fires it
                    # during STARTUP (~14us), stealing the critical window's
                    # HBM bandwidth. Sharing the tag paces each w2 trigger
                    # behind pair j-consumption (~101us+, still >10us before
                    # Phase B needs it).
                    w2sb = wp.tile([P, 2, II, P], BF16, tag="w13p", bufs=3)
                    eng = (nc.gpsimd, nc.scalar, nc.gpsimd, nc.scalar)[q]
                    eng.dma_start(w2sb[:], w2t[q])
                    yd = yp.tile([P, 2, C], BF16, tag="y2")
                halves = [(0, C)] if hh < KH - 1 or C <= TAIL else [
                    (0, C - TAIL), (C - TAIL, TAIL),
                ]
                for hi, (c0, cc) in enumerate(halves):
                    py = psB.tile([P, cc], F32, tag="py")
                    for ik in range(II):
                        nc.tensor.matmul(
                            py,
                            w2sb[:, b, ik, :],
                            psb[:, ik, c0 : c0 + cc],
                            start=(ik == 0),
                            stop=(ik == II - 1),
                        )
                    # DVE copies keep the COPY activation table off the
                    # scalar queue (its ACT_TABLE_LOAD would delay the scalar
                    # DMA ring's startup-critical triggers by ~1.3us).
                    if hh < KH - 1 or hi == 0:
                        nc.vector.tensor_copy(yd[:, b, c0 : c0 + cc], py)
                        if b == 1 and hh < KH - 1:
                            nc.sync.dma_start(yt[q], yd[:])
                        elif hh == KH - 1:
                            # penultimate store: hh=6 whole + hh=7 first
                            # piece, one trigger
                            nc.sync.dma_start(
                                yt[q, :, 0, :], yd[:, 0, :]
                            )
                            nc.sync.dma_start(
                                yt[q, :, 1, c0 : c0 + cc],
                                yd[:, 1, c0 : c0 + cc],
                            )
                    else:
                        # Final piece on the other hardware DGE ring,
                        # pipelined behind the first piece's store.
                        yb = yp.tile([P, cc], BF16, tag="y")
                        nc.vector.tensor_copy(yb, py)
                        nc.scalar.dma_start(
                            yt[q, :, 1, c0 : c0 + cc], yb[:]
                        )

    nc.compile()
    return nc


_PROGRAM_CACHE = {}


def _host_swiglu(x, w1e, w2e, w3e):
    g = x @ w1e
    u = x @ w3e
    g = g / (1.0 + np.exp(-g))
    return (g * u) @ w2e


def kernel(x, expert_indices, expert_weights, w1, w2, w3):
    x = np.asarray(x, dtype=np.float32)
    idx = np.asarray(expert_indices)
    wts = np.asarray(expert_weights, dtype=np.float32)
    w1 = np.asarray(w1, dtype=np.float32)
    w2 = np.asarray(w2, dtype=np.float32)
    w3 = np.asarray(w3, dtype=np.float32)
    N = x.shape[0]
    E = w1.shape[0]
    K = idx.shape[1]
    bf16 = ml_dtypes.bfloat16

    # host-side routing with dedup: a token whose routing slots both hit
    # expert e is computed once with the slot weights summed (exact:
    # (w0+w1)*FFN = w0*FFN + w1*FFN). Tokens beyond CAP spill to the host
    # f32 path (tiny tail, keeps device at one full-width PE chunk).
    toks, tokw, spill_toks, spill_w = [], [], [], []
    for e in range(E):
        hit = idx == e  # [N, K]
        rows = np.nonzero(hit.any(axis=1))[0]
        w_e = (wts[rows] * hit[rows]).sum(axis=1)
        toks.append(rows[:CAP])
        tokw.append(w_e[:CAP])
        spill_toks.append(rows[CAP:])
        spill_w.append(w_e[CAP:])
    C = max(16, max(len(t) for t in toks))
    C = ((C + 7) // 8) * 8

    if C not in _PROGRAM_CACHE:
        _PROGRAM_CACHE[C] = _build(C)
    nc = _PROGRAM_CACHE[C]

    in_maps = []
    for e in range(E):
        xt = np.zeros((C, H), dtype=np.float32)
        if len(toks[e]):
            xt[: len(toks[e])] = x[toks[e]]
        # [C, H] -> [hp, kh, c]
        xge = xt.T.reshape(KH, P, C).transpose(1, 0, 2)
        # w1/w3 [H, I] -> [ii, hp, {w1,w3}, kh, m]
        w13 = np.stack(
            [
                w1[e].reshape(KH, P, II, P).transpose(2, 1, 0, 3),
                w3[e].reshape(KH, P, II, P).transpose(2, 1, 0, 3),
            ],
            axis=2,
        )  # [II, P, 2, KH, P]
        w13 = w13.astype(bf16)
        # pair-major pairs for ii>=4: [II/2-2, P, 2, 2, KH, P]
        w13p = np.ascontiguousarray(
            w13[4:].reshape((II - 4) // 2, 2, P, 2, KH, P).swapaxes(1, 2)
        )
        in_maps.append(
            {
                "xg": np.ascontiguousarray(xge.astype(bf16)),
                "w013t": np.ascontiguousarray(w13[0]),
                "w13st": np.ascontiguousarray(w13[1:4]),
                "w13pt": w13p,
                "w2t": np.ascontiguousarray(
                    w2[e].reshape(II, P, KH, P).transpose(2, 1, 0, 3)
                    .reshape(KH // 2, 2, P, II, P).swapaxes(1, 2).astype(bf16)
                ),
            }
        )

    res = run_bass_kernel_spmd(nc, in_maps, core_ids=list(range(E)))

    out = np.zeros((N, H), dtype=np.float32)
    for e in range(E):
        cnt = len(toks[e])
        if cnt:
            y = (
                res.results[e]["yt"]
                .astype(np.float32)
                .reshape(KH // 2, P, 2, C)
                .swapaxes(1, 2)
                .reshape(H, C)
                .T[:cnt]
            )
            np.add.at(out, toks[e], y * tokw[e][:, None])
        if len(spill_toks[e]):
            ys = _host_swiglu(x[spill_toks[e]], w1[e], w2[e], w3[e])
            np.add.at(out, spill_toks[e], ys * spill_w[e][:, None])
    return out


# revision 21
# speedup vs baseline: 1.1956x; 1.0101x over previous
"""Expert-parallel MoE SwiGLU kernel for one TRN2 chip (8 NeuronCores).

Problem: out[n] = sum_k w[n,k] * FFN_{idx[n,k]}(x[n]) with E=8 experts,
top-2 routing, H=1024, I=4096, N=2048 tokens.

Strategy: one expert per core. Tokens are routed (gathered) per expert on
the host, each core runs the three bf16 matmuls of its expert's SwiGLU FFN
(silu(x@w1) * (x@w3)) @ w2 over its token batch entirely transposed
(tokens along the PE moving/free dim), and the host scatter-adds the
returned per-expert outputs with the routing weights. Tokens whose two
routing slots hit the SAME expert are deduplicated on the host (weights
summed, FFN computed once). Expert token counts above the per-core
capacity CAP spill to a small host-side f32 pass so the device runs a
single full-width chunk.

Schedule notes (from NTFF profile analysis): the kernel is tensor-bound
(768 bf16 matmuls of C moving columns at ~0.42 ns/col; fp8 is ruled out
by the 2e-2 relative-error budget — measured 5.7% rel err vs bf16's
0.48%). Startup: the engine preamble runs ~7.3us, then the DMA rings
start (sync ring's first bytes ~0.8us after trigger, scalar ~3.2us,
gpsimd ~4.3us). Ring arbitration under multi-way contention is
winner-take-most and varies run to run, so the early window is a strict
2-way split: sync carries the ii=0 block in its solo head start
(~11.3us), then x in two kh-halves, then pairs j0/j2; scalar carries
the ii=1..3 singles FIFO-serialized then pair j1; gpsimd stays dark
until ~30us. The ii=0/ii=1 chains run as open-psum half-chains
interleaved around the x-hi arrival so the PE goes dense from x-lo
(~13.5us) instead of waiting for all of x. Warmup matmuls ramp the HAM
clock and keep the PE busy until the gate lands (a >3us PE idle gap
triggers a HAM down-throttle to half clock that costs 5-20us). All
remaining weight traffic (1MB w13 pair tiles, then w2 pair tiles)
shares one tile-pool tag (bufs=3) whose buffer reuse paces every DMA
trigger behind pair j-3's chain consumption — without this the
scheduler fires dependency-free triggers during startup and floods the
critical window (w2 tiles were observed transferring at ~14us, 100us
before Phase B needs them). Pairs rotate gpsimd/sync/scalar from j3. y
is returned as bf16 (halves the store traffic; +0.4% quadrature error,
well within budget) and the final output chunk is split 240/240 (both
halves above the ~230-col LDWEIGHTS-bound threshold) with the last
piece on the scalar ring to shorten the kernel tail.
"""

import sys

for _p in ("/opt/trn_rl_repo", "/opt/pypackages"):
    if _p not in sys.path:
        sys.path.insert(0, _p)

import numpy as np
import ml_dtypes

import concourse.tile as tile
from concourse import bacc, mybir
from concourse.bass_utils import run_bass_kernel_spmd

P = 128
H = 1024
I = 4096
KH = H // P    # 8 contraction subtiles for the first matmuls
II = I // P    # 32 intermediate subtiles / contraction subtiles for w2
CAP = 480      # per-core token capacity (single PE moving chunk)
# PE warmup matmuls: ramp the HAM clock AND keep the tensor engine busy
# until the startup-critical DMAs are fully resident. Overshoot costs
# ~107ns/matmul; undershoot risks a HAM re-throttle that halves the
# early real matmul rate.
N_WARM = 32
W_COLS = 256   # warmup matmul moving width (finer tail granularity)
TAIL = 240     # final output piece width (short kernel tail). Keep the
               # final chain's moving width >= ~230 cols: below that the
               # 32-matmul chain goes LDWEIGHTS-bound (~97ns/matmul floor)
               # and the split costs more PE time than the tail it saves.

BF16 = mybir.dt.bfloat16
F32 = mybir.dt.float32


def _build(C):
    """One-expert SwiGLU FFN over C tokens (C <= 512), transposed layout.

    DRAM inputs (per core):
      xg    [P, KH, C]          bf16  x^T: [hp, kh, c] = x[tok c, kh*P+hp]
      w013t [P, 2, KH, P]       bf16  ii=0 block: [hp, 0, kh, m] = w1,
            [hp, 1, kh, m] = w3 (one 4KB-line transfer on scalar)
      w13st [3, P, 2, KH, P]    bf16  ii=1..3 singles (sync ring):
            [i, hp, 0, kh, m] = w1[kh*P+hp, (i+1)*P+m], [_,1,_] = w3
      w13pt [14, P, 2, 2, KH, P] bf16, pair-major ii blocks (ii=4..31):
            [j, hp, a, 0, kh, m] = w1[kh*P+hp, (2j+4+a)*P+m], [...,1,...] = w3
      w2t   [KH/2, P, 2, II, P] bf16, pair-major hh blocks:
            [q, ip, b, ik, m] = w2[ik*P+ip, (2q+b)*P+m]
    Output:
      yt    [KH/2, P, 2, C]     bf16  y^T, pair-major output subtiles
    """
    assert C <= 512
    nc = bacc.Bacc("TRN2", target_bir_lowering=False, debug=False)
    xg = nc.dram_tensor("xg", [P, KH, C], BF16, kind="ExternalInput")
    w013t = nc.dram_tensor("w013t", [P, 2, KH, P], BF16, kind="ExternalInput")
    w13st = nc.dram_tensor(
        "w13st", [3, P, 2, KH, P], BF16, kind="ExternalInput"
    )
    w13pt = nc.dram_tensor(
        "w13pt", [(II - 4) // 2, P, 2, 2, KH, P], BF16, kind="ExternalInput"
    )
    w2t = nc.dram_tensor(
        "w2t", [KH // 2, P, 2, II, P], BF16, kind="ExternalInput"
    )
    yt = nc.dram_tensor("yt", [KH // 2, P, 2, C], BF16, kind="ExternalOutput")

    with tile.TileContext(nc) as tc:
        with (
            tc.tile_pool(name="xp", bufs=2) as xp,
            tc.tile_pool(name="pp", bufs=1) as pp,
            tc.tile_pool(name="wp", bufs=8) as wp,
            tc.tile_pool(name="gp", bufs=4) as gp,
            tc.tile_pool(name="yp", bufs=3) as yp,
            tc.tile_pool(name="warm", bufs=1) as warm,
            tc.tile_pool(name="psA", bufs=3, space="PSUM") as psA,
            tc.tile_pool(name="psB", bufs=2, space="PSUM") as psB,
        ):
            # Startup-critical loads, all on sync (first ring up, ~8.7us):
            # the ii=0 weight block rides the solo window (lands ~11.3us),
            # then x in two kh-halves (~13.5/~16us) so the prologue chains
            # can start on x-lo. high_priority pins these triggers at the
            # head of the engine queue.
            with tc.high_priority():
                w013sb = wp.tile([P, 2, KH, P], BF16, tag="w0", bufs=1)
                nc.sync.dma_start(w013sb[:], w013t[:])
                xlo = xp.tile([P, KH // 2, C], BF16)
                nc.sync.dma_start(xlo[:], xg[:, : KH // 2, :])
                xhi = xp.tile([P, KH // 2, C], BF16)
                nc.sync.dma_start(xhi[:], xg[:, KH // 2 :, :])

            def xh(kh):
                return xlo[:, kh, :] if kh < KH // 2 else xhi[:, kh - KH // 2, :]

            # PE warmup: ramp the tensor engine to high-activity clock while
            # the input DMAs are in flight. Reads a zeroed tile, result is
            # never consumed.
            wtile = warm.tile([P, W_COLS], BF16)
            nc.vector.memset(wtile[:], 0.0)
            # Shares the Phase B psum pool (tag "py"): warmup is long done
            # before Phase B allocates its first chain psum.
            wps = psB.tile([P, W_COLS], F32, tag="py")
            for i in range(N_WARM):
                nc.tensor.matmul(
                    wps, wtile[:, :P], wtile[:], start=(i == 0),
                    stop=(i == N_WARM - 1),
                )

            psb = pp.tile([P, II, C], BF16)

            # Phase A: h1 = silu(x@w1), h3 = x@w3, p = h1*h3 (all transposed)
            # Prologue: ii=0 and ii=1 run as open-psum half-chains
            # interleaved around the x-hi arrival: [u0,g0,u1,g1] over
            # kh 0..3 as soon as w013+x-lo land (~12.5us), then the same
            # over kh 4..7 once x-hi lands (~15.2us). Keeps the PE dense
            # from x-lo onward instead of waiting for all of x.
            ws1 = wp.tile([P, 2, KH, P], BF16, tag="w13", bufs=3)
            nc.scalar.dma_start(ws1[:], w13st[0])
            psel = [
                lambda half, kh: w013sb[:, half, kh, :],
                lambda half, kh: ws1[:, half, kh, :],
            ]
            pg0 = psA.tile([P, C], F32, tag="pg")
            pg1 = psA.tile([P, C], F32, tag="pg")
            pu0 = psA.tile([P, C], F32, tag="pu")
            pu1 = psA.tile([P, C], F32, tag="pu")
            ppg = [pg0, pg1]
            ppu = [pu0, pu1]
            for lo in (0, KH // 2):
                for pii in (0, 1):
                    for ps, h in ((ppu[pii], 1), (ppg[pii], 0)):
                        for kh in range(lo, lo + KH // 2):
                            nc.tensor.matmul(
                                ps,
                                psel[pii](h, kh),
                                xh(kh),
                                start=(kh == 0),
                                stop=(kh == KH - 1),
                            )
            for pii in (0, 1):
                gs = gp.tile([P, C], BF16, tag="g")
                nc.scalar.activation(
                    gs, ppg[pii], mybir.ActivationFunctionType.Silu
                )
                nc.vector.tensor_tensor(
                    psb[:, pii, :], gs, ppu[pii], mybir.AluOpType.mult
                )

            wpair = None
            for ii in range(2, II):
                if ii in (2, 3):
                    # Singles ride scalar FIFO-serialized (2-way early
                    # split: sync carries w013+x+j0/j2, scalar the
                    # singles+j1; gpsimd stays dark until ~30us).
                    wsb = wp.tile([P, 2, KH, P], BF16, tag="w13", bufs=3)
                    nc.scalar.dma_start(wsb[:], w13st[ii - 1])
                    wsel = lambda half, kh, t=wsb: t[:, half, kh, :]
                elif (ii - 4) % 2 == 0:
                    j = (ii - 4) // 2
                    wpair = wp.tile([P, 2, 2, KH, P], BF16, tag="w13p", bufs=3)
                    # j0/j1 behind the ii=0 block on scalar, j2 behind the
                    # singles on sync; from j3 the pool tag paces triggers
                    # (trigger j waits until pair j-3's chains complete) and
                    # the three rings rotate.
                    if j < 3:
                        # all three early pairs behind x on sync: keeps
                        # scalar's early pull to just the 1.5MB of singles
                        # so the sync gate stream (w013+x halves) gets the
                        # larger HBM share.
                        eng = nc.sync
                    else:
                        eng = (nc.gpsimd, nc.scalar, nc.sync)[(j - 3) % 3]
                    eng.dma_start(wpair[:], w13pt[j])
                    wsel = lambda half, kh, t=wpair: t[:, 0, half, kh, :]
                else:
                    wsel = lambda half, kh, t=wpair: t[:, 1, half, kh, :]
                pg = psA.tile([P, C], F32, tag="pg")
                pu = psA.tile([P, C], F32, tag="pu")
                halves = (0, 1)
                for half in halves:
                    ps = pg if half == 0 else pu
                    for kh in range(KH):
                        nc.tensor.matmul(
                            ps,
                            wsel(half, kh),
                            xh(kh),
                            start=(kh == 0),
                            stop=(kh == KH - 1),
                        )
                gs = gp.tile([P, C], BF16, tag="g")
                nc.scalar.activation(gs, pg, mybir.ActivationFunctionType.Silu)
                nc.vector.tensor_tensor(
                    psb[:, ii, :], gs, pu, mybir.AluOpType.mult
                )

            # Phase B: y = p @ w2 (transposed: yT = w2T-contraction over I).
            # w2 arrives as 1MB pair tiles (hh 2q, 2q+1 together) rotated
            # across gpsimd/sync/scalar (gpsimd's trigger stream runs ahead
            # of compute, so q=0 prefetches early) and y goes out as bf16
            # pair stores. The last hh is split column-wise so its first
            # piece's copy+DMA overlaps the final piece's matmuls (shorter
            # kernel tail), with the final small piece on the scalar ring.
            w2sb = yd = None
            for hh in range(KH):
                q, b = divmod(hh, 2)
                if b == 0:
                    # w2 tiles share the pairs' pool tag: a bare-pool w2
                    # trigger has no dependencies and the scheduler fires it
                    # during STARTUP (~14us), stealing the critical window's
                    # HBM bandwidth. Sharing the tag paces each w2 trigger
                    # behind pair j-consumption (~101us+, still >10us before
                    # Phase B needs it).
                    w2sb = wp.tile([P, 2, II, P], BF16, tag="w13p", bufs=3)
                    eng = (nc.gpsimd, nc.scalar, nc.gpsimd, nc.scalar)[q]
                    eng.dma_start(w2sb[:], w2t[q])
                    yd = yp.tile([P, 2, C], BF16, tag="y2")
                halves = [(0, C)] if hh < KH - 1 or C <= TAIL else [
                    (0, C - TAIL), (C - TAIL, TAIL),
                ]
                for hi, (c0, cc) in enumerate(halves):
                    py = psB.tile([P, cc], F32, tag="py")
                    for ik in range(II):
                        nc.tensor.matmul(
                            py,
                            w2sb[:, b, ik, :],
                            psb[:, ik, c0 : c0 + cc],
                            start=(ik == 0),
                            stop=(ik == II - 1),
                        )
                    # DVE copies keep the COPY activation table off the
                    # scalar queue (its ACT_TABLE_LOAD would delay the scalar
                    # DMA ring's startup-critical triggers by ~1.3us).
                    if hh < KH - 1 or hi == 0:
                        nc.vector.tensor_copy(yd[:, b, c0 : c0 + cc], py)
                        if b == 1 and hh < KH - 1:
                            nc.sync.dma_start(yt[q], yd[:])
                        elif hh == KH - 1:
                            # penultimate store: hh=6 whole + hh=7 first
                            # piece, one trigger
                            nc.sync.dma_start(
                                yt[q, :, 0, :], yd[:, 0, :]
                            )
                            nc.sync.dma_start(
                                yt[q, :, 1, c0 : c0 + cc],
                                yd[:, 1, c0 : c0 + cc],
                            )
                    else:
                        # Final piece on the other hardware DGE ring,
                        # pipelined behind the first piece's store.
                        yb = yp.tile([P, cc], BF16, tag="y")
                        nc.vector.tensor_copy(yb, py)
                        nc.scalar.dma_start(
                            yt[q, :, 1, c0 : c0 + cc], yb[:]
                        )

    nc.compile()
    return nc


_PROGRAM_CACHE = {}


def _host_swiglu(x, w1e, w2e, w3e):
    g = x @ w1e
    u = x @ w3e
    g = g / (1.0 + np.exp(-g))
    return (g * u) @ w2e


def kernel(x, expert_indices, expert_weights, w1, w2, w3):
    x = np.asarray(x, dtype=np.float32)
    idx = np.asarray(expert_indices)
    wts = np.asarray(expert_weights, dtype=np.float32)
    w1 = np.asarray(w1, dtype=np.float32)
    w2 = np.asarray(w2, dtype=np.float32)
    w3 = np.asarray(w3, dtype=np.float32)
    N = x.shape[0]
    E = w1.shape[0]
    K = idx.shape[1]
    bf16 = ml_dtypes.bfloat16

    # host-side routing with dedup: a token whose routing slots both hit
    # expert e is computed once with the slot weights summed (exact:
    # (w0+w1)*FFN = w0*FFN + w1*FFN). Tokens beyond CAP spill to the host
    # f32 path (tiny tail, keeps device at one full-width PE chunk).
    toks, tokw, spill_toks, spill_w = [], [], [], []
    for e in range(E):
        hit = idx == e  # [N, K]
        rows = np.nonzero(hit.any(axis=1))[0]
        w_e = (wts[rows] * hit[rows]).sum(axis=1)
        toks.append(rows[:CAP])
        tokw.append(w_e[:CAP])
        spill_toks.append(rows[CAP:])
        spill_w.append(w_e[CAP:])
    C = max(16, max(len(t) for t in toks))
    C = ((C + 7) // 8) * 8

    if C not in _PROGRAM_CACHE:
        _PROGRAM_CACHE[C] = _build(C)
    nc = _PROGRAM_CACHE[C]

    in_maps = []
    for e in range(E):
        xt = np.zeros((C, H), dtype=np.float32)
        if len(toks[e]):
            xt[: len(toks[e])] = x[toks[e]]
        # [C, H] -> [hp, kh, c]
        xge = xt.T.reshape(KH, P, C).transpose(1, 0, 2)
        # w1/w3 [H, I] -> [ii, hp, {w1,w3}, kh, m]
        w13 = np.stack(
            [
                w1[e].reshape(KH, P, II, P).transpose(2, 1, 0, 3),
                w3[e].reshape(KH, P, II, P).transpose(2, 1, 0, 3),
            ],
            axis=2,
        )  # [II, P, 2, KH, P]
        w13 = w13.astype(bf16)
        # pair-major pairs for ii>=4: [II/2-2, P, 2, 2, KH, P]
        w13p = np.ascontiguousarray(
            w13[4:].reshape((II - 4) // 2, 2, P, 2, KH, P).swapaxes(1, 2)
        )
        in_maps.append(
            {
                "xg": np.ascontiguousarray(xge.astype(bf16)),
                "w013t": np.ascontiguousarray(w13[0]),
                "w13st": np.ascontiguousarray(w13[1:4]),
                "w13pt": w13p,
                "w2t": np.ascontiguousarray(
                    w2[e].reshape(II, P, KH, P).transpose(2, 1, 0, 3)
                    .reshape(KH // 2, 2, P, II, P).swapaxes(1, 2).astype(bf16)
                ),
            }
        )

    res = run_bass_kernel_spmd(nc, in_maps, core_ids=list(range(E)))

    out = np.zeros((N, H), dtype=np.float32)
    for e in range(E):
        cnt = len(toks[e])
        if cnt:
            y = (
                res.results[e]["yt"]
                .astype(np.float32)
                .reshape(KH // 2, P, 2, C)
                .swapaxes(1, 2)
                .reshape(H, C)
                .T[:cnt]
            )
            np.add.at(out, toks[e], y * tokw[e][:, None])
        if len(spill_toks[e]):
            ys = _host_swiglu(x[spill_toks[e]], w1[e], w2[e], w3[e])
            np.add.at(out, spill_toks[e], ys * spill_w[e][:, None])
    return out
